# revision 1
# baseline (speedup 1.0000x reference)
"""GAT layer (nn_GATLayer) on 8 Trainium2 NeuronCores.

Sharding: edges+output nodes sharded by dst-node range (edge-cut, as in the
hint); node features (fc projection) computed replicated on every core so the
per-edge gather is purely local. All FP compute on device; host only does
integer graph partitioning / index-table construction and final concat.

Per core (dst nodes [k*2500, (k+1)*2500)):
  Phase 1 (all N nodes, replicated): feat = x @ W_fc.T (+b), el/er attention
    dots folded into the same matmul via precomposed [W_fc.T | W_el | W_er].
    Writes a gatherable table featx[n] = [feat_b0, feat_b1, el_b0, el_b1, pad]
    (1280B rows) and er table (256B rows) to DRAM.
  Phase 2: dst-sorted edges in groups of 128 dst nodes, chunks of 128 edges.
    dma_gather pulls feat+el rows for edge sources; indicator matrices
    S[e,m] = (lid[e]==m) built on-device from per-edge local-dst ids give
    segment ops as PE matmuls: er[dst] expand, softmax-denominator
    accumulate, 1/s expand, and the weighted scatter-add aggregation.
    Softmax computed without max-subtraction (logits bounded: |(el+er)*w|
    << 80, exp cannot overflow in fp32).
  Epilogue per group: agg -> transpose -> block-diag W_out matmul -> out.
"""

import numpy as np
from contextlib import ExitStack

import concourse.bass as bass
import concourse.bacc as bacc
import concourse.tile as tile
from concourse import mybir
from concourse.bass_utils import run_bass_kernel_spmd
from concourse.masks import make_identity

B, N, D, H, DH, OUT = 2, 20000, 128, 8, 16, 64
E = 320000
NEG_SLOPE = 0.1
NCORES = 8
NPC = N // NCORES            # 2500 dst nodes per core
NG = (NPC + 127) // 128      # 20 groups of <=128 dst nodes
ROW_F = 320                  # featx row: 256 feat + 16 el + 48 pad (1280B)
F32 = mybir.dt.float32
I16 = mybir.dt.int16
I32 = mybir.dt.int32
EQ = mybir.AluOpType.is_equal
MULT = mybir.AluOpType.mult
MAX = mybir.AluOpType.max

LAST_RESULTS = None  # test harness can inspect exec_time_ns / profile


def _bcast_free(ap, n):
    """[P,1] AP -> [P,n] via step-0 free dim."""
    return bass.AP(tensor=ap.tensor, offset=ap.offset, ap=[ap.ap[0], [0, n]])


def _bcast_inner(ap, n):
    """[P,c] AP -> [P,c,n] via step-0 innermost dim."""
    return bass.AP(
        tensor=ap.tensor, offset=ap.offset, ap=[ap.ap[0], ap.ap[1], [0, n]]
    )


def _build_program(MAXC):
    nc = bacc.Bacc(
        "TRN2", target_bir_lowering=False, debug=False, num_devices=NCORES
    )
    x_d = nc.dram_tensor("x", [B * N, D], F32, kind="ExternalInput").ap()
    wcat_d = nc.dram_tensor("wcat", [D, 144], F32, kind="ExternalInput").ap()
    bcat_d = nc.dram_tensor("bcat", [128, 144], F32, kind="ExternalInput").ap()
    wblk_d = nc.dram_tensor("wblk", [D, 512], F32, kind="ExternalInput").ap()
    bout_d = nc.dram_tensor("bout", [128, 512], F32, kind="ExternalInput").ap()
    gidx_d = nc.dram_tensor(
        "gidx", [128, NG * MAXC * 8], I16, kind="ExternalInput"
    ).ap()
    ernidx_d = nc.dram_tensor("ernidx", [128, NG * 8], I16, kind="ExternalInput").ap()
    lid_d = nc.dram_tensor("lid", [128, NG * MAXC], F32, kind="ExternalInput").ap()
    wcol_d = nc.dram_tensor("wcol", [128, NG * MAXC], F32, kind="ExternalInput").ap()
    out_d = nc.dram_tensor("out", [B, NPC, 512], F32, kind="ExternalOutput").ap()

    with ExitStack() as ctx:
        tc = ctx.enter_context(tile.TileContext(nc))
        dram = ctx.enter_context(tc.tile_pool(name="dram", bufs=1, space="DRAM"))
        featx = dram.tile([N, ROW_F], F32)
        ertab = dram.tile([N, 64], F32)

        singles = ctx.enter_context(tc.tile_pool(name="singles", bufs=1))
        ident = singles.tile([128, 128], F32)
        make_identity(nc, ident)
        iota_i = singles.tile([128, 128], I32)
        nc.gpsimd.iota(iota_i, pattern=[[1, 128]], base=0, channel_multiplier=0)
        iota_f = singles.tile([128, 128], F32)
        nc.vector.tensor_copy(iota_f, iota_i)

        wcat_sb = singles.tile([128, 144], F32)
        nc.sync.dma_start(wcat_sb, wcat_d)
        bcat_sb = singles.tile([128, 144], F32)
        nc.sync.dma_start(bcat_sb, bcat_d)
        wblk_sb = singles.tile([128, 512], F32)
        nc.sync.dma_start(wblk_sb, wblk_d)
        bout_sb = singles.tile([128, 512], F32)
        nc.sync.dma_start(bout_sb, bout_d)
        gidx_sb = singles.tile([128, NG * MAXC * 8], I16)
        nc.sync.dma_start(gidx_sb, gidx_d)
        ernidx_sb = singles.tile([128, NG * 8], I16)
        nc.sync.dma_start(ernidx_sb, ernidx_d)
        lid_sb = singles.tile([128, NG * MAXC], F32)
        nc.sync.dma_start(lid_sb, lid_d)
        wcol_sb = singles.tile([128, NG * MAXC], F32)
        nc.sync.dma_start(wcol_sb, wcol_d)

        # ---------------- Phase 1: feat/el/er for all N nodes ----------------
        with tile.contextlib.ExitStack() if False else ExitStack() as p1ctx:
            p1x = p1ctx.enter_context(tc.tile_pool(name="p1x", bufs=4))
            p1t = p1ctx.enter_context(tc.tile_pool(name="p1t", bufs=4))
            p1o = p1ctx.enter_context(tc.tile_pool(name="p1o", bufs=4))
            p1psT = p1ctx.enter_context(tc.tile_pool(name="p1psT", bufs=2, space="PSUM"))
            p1psM = p1ctx.enter_context(tc.tile_pool(name="p1psM", bufs=2, space="PSUM"))
            NT = (N + 127) // 128
            for b in range(B):
                for t in range(NT):
                    n0 = t * 128
                    rows = min(128, N - n0)
                    x_sb = p1x.tile([128, D], F32)
                    nc.sync.dma_start(
                        x_sb[:rows], x_d[b * N + n0 : b * N + n0 + rows, :]
                    )
                    xT_ps = p1psT.tile([128, 128], F32)
                    nc.tensor.transpose(
                        xT_ps[:, :rows], x_sb[:rows], ident[:rows, :rows]
                    )
                    xT_sb = p1t.tile([128, 128], F32)
                    nc.vector.tensor_copy(xT_sb[:, :rows], xT_ps[:, :rows])
                    f_ps = p1psM.tile([128, 144], F32)
                    nc.tensor.matmul(
                        f_ps[:rows], xT_sb[:, :rows], wcat_sb, start=True, stop=True
                    )
                    f_sb = p1o.tile([128, 144], F32)
                    nc.vector.tensor_add(f_sb[:rows], f_ps[:rows], bcat_sb[:rows])
                    nc.sync.dma_start(
                        featx[n0 : n0 + rows, b * 128 : (b + 1) * 128],
                        f_sb[:rows, 0:128],
                    )
                    nc.sync.dma_start(
                        featx[n0 : n0 + rows, 256 + b * 8 : 256 + b * 8 + 8],
                        f_sb[:rows, 128:136],
                    )
                    nc.sync.dma_start(
                        ertab[n0 : n0 + rows, b * 8 : b * 8 + 8],
                        f_sb[:rows, 136:144],
                    )

        # ---------------- Phase 2: per-edge softmax + aggregation ------------
        p2g = ctx.enter_context(tc.tile_pool(name="p2g", bufs=2))
        p2er = ctx.enter_context(tc.tile_pool(name="p2er", bufs=2))
        p2S = ctx.enter_context(tc.tile_pool(name="p2S", bufs=2 * MAXC + 2))
        p2ST = ctx.enter_context(tc.tile_pool(name="p2ST", bufs=2 * MAXC + 2))
        p2ex = ctx.enter_context(tc.tile_pool(name="p2ex", bufs=2 * MAXC + 2))
        p2sm = ctx.enter_context(tc.tile_pool(name="p2sm", bufs=8))
        p2inv = ctx.enter_context(tc.tile_pool(name="p2inv", bufs=2))
        p2msg = ctx.enter_context(tc.tile_pool(name="p2msg", bufs=4))
        p2fin = ctx.enter_context(tc.tile_pool(name="p2fin", bufs=4))
        pp_stp = ctx.enter_context(tc.tile_pool(name="pp_stp", bufs=2, space="PSUM"))
        pp_e = ctx.enter_context(tc.tile_pool(name="pp_e", bufs=2, space="PSUM"))
        pp_s = ctx.enter_context(tc.tile_pool(name="pp_s", bufs=1, space="PSUM"))
        pp_agg = ctx.enter_context(tc.tile_pool(name="pp_agg", bufs=1, space="PSUM"))
        pp_T = ctx.enter_context(tc.tile_pool(name="pp_T", bufs=1, space="PSUM"))
        pp_r = ctx.enter_context(tc.tile_pool(name="pp_r", bufs=1, space="PSUM"))

        for g in range(NG):
            rows_g = min(128, NPC - g * 128)

            # gather er rows for this group's dst nodes (per-core node ids)
            ern_sb = p2er.tile([128, 1, 64], F32)
            nc.gpsimd.dma_gather(
                out_ap=ern_sb[:],
                in_ap=ertab[:, :],
                idxs_ap=ernidx_sb[:, g * 8 : (g + 1) * 8],
                num_idxs=128,
                num_idxs_reg=128,
                elem_size=64,
            )
            er16 = ern_sb[:, 0, 0:16]

            # gather feat+el rows for this group's edge sources
            g_all = p2g.tile([128, MAXC, ROW_F], F32)
            for cc in range(0, MAXC, 4):
                sz = min(4, MAXC - cc)
                nc.gpsimd.dma_gather(
                    out_ap=g_all[:, cc : cc + sz, :],
                    in_ap=featx[:, :],
                    idxs_ap=gidx_sb[
                        :, (g * MAXC + cc) * 8 : (g * MAXC + cc + sz) * 8
                    ],
                    num_idxs=sz * 128,
                    num_idxs_reg=sz * 128,
                    elem_size=ROW_F,
                )

            S_list, ST_list, ex_list = [], [], []
            s_ps = pp_s.tile([128, 16], F32)
            for c in range(MAXC):
                gc = g * MAXC + c
                # S[e, m] = (lid[e] == m)
                S_sb = p2S.tile([128, 128], F32, tag="S")
                nc.vector.tensor_tensor(
                    S_sb, _bcast_free(lid_sb[:, gc : gc + 1], 128), iota_f, EQ
                )
                ST_ps = pp_stp.tile([128, 128], F32, tag="stp")
                nc.tensor.transpose(ST_ps, S_sb, ident)
                ST_sb = p2ST.tile([128, 128], F32, tag="ST")
                nc.vector.tensor_copy(ST_sb, ST_ps)
                # er[dst] expand: [e,16] = S_T.T @ er16
                er_ps = pp_e.tile([128, 16], F32, tag="erp")
                nc.tensor.matmul(er_ps, ST_sb, er16, start=True, stop=True)
                # logit = leaky((el_src + er_dst) * w)
                t0 = p2sm.tile([128, 16], F32, tag="t0")
                nc.vector.tensor_add(t0, g_all[:, c, 256:272], er_ps)
                t1 = p2sm.tile([128, 16], F32, tag="t1")
                nc.vector.tensor_scalar_mul(t1, t0, wcol_sb[:, gc : gc + 1])
                t2 = p2sm.tile([128, 16], F32, tag="t2")
                nc.vector.scalar_tensor_tensor(t2, t1, NEG_SLOPE, t1, MULT, MAX)
                ex_sb = p2ex.tile([128, 16], F32, tag="ex")
                nc.scalar.activation(ex_sb, t2, mybir.ActivationFunctionType.Exp)
                # s[m] += S.T @ ex
                nc.tensor.matmul(
                    s_ps, S_sb, ex_sb, start=(c == 0), stop=(c == MAXC - 1)
                )
                S_list.append(S_sb)
                ST_list.append(ST_sb)
                ex_list.append(ex_sb)

            inv_sb = p2inv.tile([128, 16], F32)
            nc.vector.tensor_scalar_add(inv_sb, s_ps, 1e-30)
            nc.vector.reciprocal(inv_sb, inv_sb)

            agg_ps = pp_agg.tile([128, 256], F32)
            for c in range(MAXC):
                ainv_ps = pp_e.tile([128, 16], F32, tag="erp")
                nc.tensor.matmul(ainv_ps, ST_list[c], inv_sb, start=True, stop=True)
                a_sb = p2sm.tile([128, 16], F32, tag="a")
                nc.vector.tensor_mul(a_sb, ex_list[c], ainv_ps)
                msg_sb = p2msg.tile([128, 16, 16], F32)
                nc.vector.tensor_mul(
                    msg_sb,
                    g_all[:, c, 0:256].rearrange("p (a b) -> p a b", a=16),
                    _bcast_inner(a_sb[:, :], 16),
                )
                nc.tensor.matmul(
                    agg_ps,
                    S_list[c],
                    msg_sb.rearrange("p a b -> p (a b)"),
                    start=(c == 0),
                    stop=(c == MAXC - 1),
                )

            agg_sb = p2fin.tile([128, 256], F32, tag="agg")
            nc.vector.tensor_copy(agg_sb, agg_ps)
            for b in range(B):
                aggT_ps = pp_T.tile([128, 128], F32)
                nc.tensor.transpose(
                    aggT_ps, agg_sb[:, b * 128 : (b + 1) * 128], ident
                )
                aggT_sb = p2fin.tile([128, 128], F32, tag="aggT")
                nc.vector.tensor_copy(aggT_sb, aggT_ps)
                rst_ps = pp_r.tile([128, 512], F32)
                nc.tensor.matmul(rst_ps, aggT_sb, wblk_sb, start=True, stop=True)
                rst_sb = p2fin.tile([128, 512], F32, tag="rst")
                nc.vector.tensor_add(rst_sb, rst_ps, bout_sb)
                nc.sync.dma_start(
                    out_d[b, g * 128 : g * 128 + rows_g, :], rst_sb[:rows_g]
                )
    nc.finalize()
    return nc


def _prep_host(x, src, dst, w, W_fc, b_fc, attn_l, attn_r, W_out, b_out):
    x = np.ascontiguousarray(np.asarray(x, np.float32).reshape(B * N, D))
    src = np.asarray(src).astype(np.int64)
    dst = np.asarray(dst).astype(np.int64)
    w = np.asarray(w, np.float32)
    W_fc = np.asarray(W_fc, np.float32)
    b_fc = np.asarray(b_fc, np.float32)
    al = np.asarray(attn_l, np.float32).reshape(H, DH)
    ar = np.asarray(attn_r, np.float32).reshape(H, DH)
    W_out = np.asarray(W_out, np.float32)
    b_out = np.asarray(b_out, np.float32)

    WfcT = np.ascontiguousarray(W_fc.T)                      # (d, e)
    W_el = np.einsum("dhk,hk->dh", WfcT.reshape(D, H, DH), al)
    W_er = np.einsum("dhk,hk->dh", WfcT.reshape(D, H, DH), ar)
    wcat = np.concatenate([WfcT, W_el, W_er], axis=1).astype(np.float32)  # (128,144)
    bel = np.einsum("hk,hk->h", b_fc.reshape(H, DH), al)
    ber = np.einsum("hk,hk->h", b_fc.reshape(H, DH), ar)
    bcat = np.tile(
        np.concatenate([b_fc, bel, ber]).astype(np.float32), (128, 1)
    )                                                         # (128,144)
    wblk = np.zeros((D, 512), np.float32)
    for h in range(H):
        wblk[h * DH : (h + 1) * DH, h * OUT : (h + 1) * OUT] = W_out.T
    bout = np.tile(np.tile(b_out, H).astype(np.float32), (128, 1))  # (128,512)

    order = np.argsort(dst, kind="stable")
    dsts, srcs, ws = dst[order], src[order], w[order]

    # group boundaries per (core, group)
    bounds = np.zeros((NCORES, NG + 1), np.int64)
    cnts = np.zeros((NCORES, NG), np.int64)
    for k in range(NCORES):
        for g in range(NG):
            lo = k * NPC + g * 128
            hi = k * NPC + min(NPC, (g + 1) * 128)
            bounds[k, g] = np.searchsorted(dsts, lo)
            bounds[k, g + 1] = np.searchsorted(dsts, hi)
            cnts[k, g] = bounds[k, g + 1] - bounds[k, g]
    MAXC = int(np.max((cnts + 127) // 128))

    gidx = np.zeros((NCORES, 128, NG * MAXC * 8), np.int16)
    ernidx = np.zeros((NCORES, 128, NG * 8), np.int16)
    lid = np.full((NCORES, 128, NG * MAXC), -1.0, np.float32)
    wcol = np.zeros((NCORES, 128, NG * MAXC), np.float32)
    for k in range(NCORES):
        for g in range(NG):
            i0, i1 = bounds[k, g], bounds[k, g + 1]
            cnt = int(i1 - i0)
            s = np.arange(cnt)
            lid[k, s % 128, g * MAXC + s // 128] = (
                dsts[i0:i1] - (k * NPC + g * 128)
            ).astype(np.float32)
            wcol[k, s % 128, g * MAXC + s // 128] = ws[i0:i1]
            gidx[k, s % 16, g * MAXC * 8 + s // 16] = srcs[i0:i1].astype(np.int16)
            rows_g = min(128, NPC - g * 128)
            i = np.arange(rows_g)
            ernidx[k, i % 16, g * 8 + i // 16] = (k * NPC + g * 128 + i).astype(
                np.int16
            )
    # q7 gather firmware reads a per-core copy of the wrapped idx block:
    # replicate partitions 0:16 across all 8 groups of 16 partitions
    gidx = np.tile(gidx[:, :16, :], (1, 8, 1))
    ernidx = np.tile(ernidx[:, :16, :], (1, 8, 1))
    return (
        x, wcat, bcat, wblk, bout, gidx, ernidx, lid, wcol, MAXC,
    )


def kernel(vt=None, x=None, src=None, dst=None, w=None, W_fc=None, b_fc=None,
           attn_l=None, attn_r=None, W_out=None, b_out=None, **_ignored):
    global LAST_RESULTS
    (x_f, wcat, bcat, wblk, bout, gidx, ernidx, lid, wcol, MAXC) = _prep_host(
        x, src, dst, w, W_fc, b_fc, attn_l, attn_r, W_out, b_out
    )
    nc = _build_program(MAXC)
    in_maps = []
    for k in range(NCORES):
        in_maps.append(
            dict(
                x=x_f,
                wcat=wcat,
                bcat=bcat,
                wblk=wblk,
                bout=bout,
                gidx=np.ascontiguousarray(gidx[k]),
                ernidx=np.ascontiguousarray(ernidx[k]),
                lid=np.ascontiguousarray(lid[k]),
                wcol=np.ascontiguousarray(wcol[k]),
            )
        )
    res = run_bass_kernel_spmd(nc, in_maps, core_ids=list(range(NCORES)))
    LAST_RESULTS = res
    import os, time
    reps = int(os.environ.get("KERNEL_TIME_REPS", "0"))
    if reps:
        times = []
        for _ in range(reps):
            t0 = time.perf_counter()
            run_bass_kernel_spmd(nc, in_maps, core_ids=list(range(NCORES)))
            times.append(time.perf_counter() - t0)
        print("repeat walls (s):", [round(t, 4) for t in times])
        print("best repeat wall: %.1f us" % (min(times) * 1e6))
    outs = [res.results[k]["out"] for k in range(NCORES)]
    full = np.concatenate(outs, axis=1)  # (B, N, 512)
    return np.ascontiguousarray(full.reshape(B, N, H, OUT))

